# revision 6
# baseline (speedup 1.0000x reference)
"""DigitCaps dynamic-routing kernel for 8 Trainium2 NeuronCores.

x[B=64, R=6912, I=8] f32, route_weights[C=10, R=6912, I=8, O=16] f32.
priors = einsum('bri,crio->cbro'); 3 routing iterations; out [C,B,1,1,O].

Sharding: data-parallel over B (8 b's per core), route_weights replicated.
Device math in f16 (validated max-rel err ~4e-4 vs f32 reference):
- Host prepacks W -> W_prep[t=432, (r',i)=128, (c,o)=160] f16 tiles and
  x -> x_prep[(r',i)=128, t, b=8] f16 per core.
- PE generates priors via block-diagonalized x (lhsT [K=(r',i), M=(r'',b)],
  rhs = W tile) -> priors f16 SBUF-resident [p=(r'',b), (t,c,o)].
- Routing fully on-chip: delta = sum_o priors*out (DVE bcast-mul+reduce),
  logits += delta, e = exp (ACT), t = sum_r e*priors (DVE bcast-mul + PE
  selector-matmul accumulating over tiles), squash on [8,(c,o)].
"""

import os
import sys

import numpy as np

sys.path.insert(0, "/opt/trn_rl_repo")

B, R, I, C, O = 64, 6912, 8, 10, 16
RP = 16            # routes packed per tile
T = R // RP        # 432 tiles
NB = 8             # batch per core
CO = C * O         # 160
TC = 16            # W-chunk tiles (gen phase)
GC = 4             # gen tiles per PSUM copy group (4*160*4B = 2560B > bank!) -> use 3
GC = 3
TC2 = 16           # boundary chunk tiles (432 = 27*16)
NCH2 = T // TC2

_CACHE = {}


def _build_program():
    import concourse.mybir as mybir
    import concourse.tile as tile
    from concourse import bacc

    f16 = mybir.dt.float16
    f32 = mybir.dt.float32
    Alu = mybir.AluOpType
    Act = mybir.ActivationFunctionType
    X = mybir.AxisListType.X

    nc = bacc.Bacc()

    w_d = nc.dram_tensor("w", [T, 128, CO], f16, kind="ExternalInput")
    x_d = nc.dram_tensor("x", [128, T, NB], f16, kind="ExternalInput")
    out_d = nc.dram_tensor("out", [NB, CO], f32, kind="ExternalOutput")

    p_idx = np.arange(128)
    mask_np = (p_idx[:, None] // 8 == p_idx[None, :] // 8).astype(np.float16)
    sel8_np = (p_idx[:, None] % 8 == np.arange(8)[None, :]).astype(np.float16)
    selB_np = (np.arange(8)[:, None] == p_idx[None, :] % 8).astype(np.float16)

    mask_d = nc.inline_tensor(mask_np, "mask")
    sel8_d = nc.inline_tensor(sel8_np, "sel8")
    sel8f_d = nc.inline_tensor(sel8_np.astype(np.float32), "sel8f")
    selB_d = nc.inline_tensor(selB_np, "selB")

    with (
        tile.TileContext(nc) as tc,
        tc.tile_pool(name="consts", bufs=1) as consts,
        tc.tile_pool(name="big", bufs=1) as big,
        tc.tile_pool(name="wpool", bufs=2) as wpool,
        tc.tile_pool(name="dpool", bufs=2) as dpool,
        tc.tile_pool(name="ppool", bufs=2) as ppool,
        tc.tile_pool(name="psum", bufs=4, space="PSUM") as psum,
        tc.tile_pool(name="psum1", bufs=1, space="PSUM") as psum1,
        tc.tile_pool(name="small", bufs=1) as small,
    ):
        mask = consts.tile([128, 128], f16)
        sel8 = consts.tile([128, 8], f16)
        sel8f = consts.tile([128, 8], f32)
        selB = consts.tile([8, 128], f16)
        nc.sync.dma_start(out=mask, in_=mask_d[:, :])
        nc.sync.dma_start(out=sel8, in_=sel8_d[:, :])
        nc.sync.dma_start(out=sel8f, in_=sel8f_d[:, :])
        nc.sync.dma_start(out=selB, in_=selB_d[:, :])

        x_sb = big.tile([128, T, NB], f16)
        nc.sync.dma_start(out=x_sb, in_=x_d[:, :, :])

        # Prime the DVE vector clock one DMA at a time: the DVE TensorTensor
        # instruction encoding has a single sync-wait slot, so no real op may
        # carry two fresh cross-engine waits.
        touch = consts.tile([128, 2], f16, tag="touch")
        nc.vector.tensor_copy(touch[:, 0:1], mask[:, 0:1])
        nc.vector.tensor_copy(touch[:, 1:2], x_sb[:, 0, 0:1])

        priors = big.tile([128, T, C, O], f16)
        logits = big.tile([128, T, C], f32)
        e_sb = big.tile([128, T, C], f16)
        nc.vector.memset(logits, 0.0)

        mask_v = mask.rearrange("p (r b) -> p r b", b=NB)

        # ---------------- generation + t0 ----------------
        ps_t0 = psum1.tile([8, CO], f32, tag="t0")
        for ch in range(T // TC):
            t0 = ch * TC
            w_t = wpool.tile([128, TC, CO], f16, tag="w")
            nc.sync.dma_start(
                out=w_t, in_=w_d[t0 : t0 + TC, :, :].transpose([1, 0, 2])
            )
            xd_t = dpool.tile([128, TC, RP, NB], f16, tag="xd")
            nc.vector.tensor_mul(
                xd_t,
                mask_v[:, None, :, :].broadcast_to([128, TC, RP, NB]),
                x_sb[:, t0 : t0 + TC, None, :].broadcast_to([128, TC, RP, NB]),
            )
            for g in range(TC // GC):
                ps_g = psum.tile([128, GC * CO], f32, tag="gen")
                for k in range(GC):
                    tl = g * GC + k
                    t = t0 + tl
                    nc.tensor.matmul(
                        ps_g[:, k * CO : (k + 1) * CO],
                        xd_t[:, tl, :, :].rearrange("p r b -> p (r b)"),
                        w_t[:, tl, :],
                        start=True,
                        stop=True,
                    )
                    nc.tensor.matmul(
                        ps_t0,
                        x_sb[:, t, :],
                        w_t[:, tl, :],
                        start=(t == 0),
                        stop=(t == T - 1),
                        skip_group_check=True,
                    )
                nc.vector.tensor_copy(
                    priors[:, t0 + g * GC : t0 + (g + 1) * GC, :, :].rearrange(
                        "p a c o -> p (a c o)"
                    ),
                    ps_g,
                )
            # leftover tile (TC=16, GC=3 -> 5 groups of 3 + 1)
            for tl in range(15, TC):
                t = t0 + tl
                ps_g = psum.tile([128, GC * CO], f32, tag="gen")
                nc.tensor.matmul(
                    ps_g[:, :CO],
                    xd_t[:, tl, :, :].rearrange("p r b -> p (r b)"),
                    w_t[:, tl, :],
                    start=True,
                    stop=True,
                )
                nc.tensor.matmul(
                    ps_t0,
                    x_sb[:, t, :],
                    w_t[:, tl, :],
                    start=(t == 0),
                    stop=(t == T - 1),
                    skip_group_check=True,
                )
                nc.vector.tensor_copy(
                    priors[:, t, :, :].rearrange("p c o -> p (c o)"),
                    ps_g[:, :CO],
                )

        # small tiles (8 partitions)
        invz = small.tile([8, C], f32)
        s_sb = small.tile([8, C, O], f32)
        tmp_co = small.tile([8, C, O], f32)
        sq = small.tile([8, C], f32)
        rt = small.tile([8, C], f32)
        onep = small.tile([8, C], f32)
        z_sb = small.tile([8, C], f32)
        out_sb = small.tile([8, C, O], f32)
        out16 = small.tile([8, C, O], f16)
        outrep = small.tile([128, C, O], f16)
        zpart = small.tile([128, C], f32)
        t_sb = small.tile([8, CO], f32)

        def squash(t_vec):
            # t_vec: [8, CO] f32 AP; uses z_sb; leaves result in out_sb/outrep
            nc.vector.reciprocal(invz, z_sb)
            nc.vector.tensor_mul(
                s_sb,
                t_vec.rearrange("p (c o) -> p c o", c=C),
                invz[:, :, None].broadcast_to([8, C, O]),
            )
            nc.vector.tensor_mul(tmp_co, s_sb, s_sb)
            nc.vector.tensor_reduce(sq, tmp_co, axis=X, op=Alu.add)
            nc.scalar.activation(rt, sq, Act.Sqrt)
            nc.vector.tensor_scalar(
                out=onep, in0=sq, scalar1=1.0, scalar2=None, op0=Alu.add
            )
            nc.vector.tensor_mul(onep, onep, rt)
            nc.vector.reciprocal(rt, onep)
            nc.vector.tensor_mul(rt, rt, sq)
            nc.vector.tensor_mul(
                out_sb, s_sb, rt[:, :, None].broadcast_to([8, C, O])
            )
            nc.vector.tensor_copy(out16, out_sb)
            ps_rep = psum1.tile([128, CO], f32, tag="rep")
            nc.tensor.matmul(ps_rep, selB, out16.rearrange("p c o -> p (c o)"))
            nc.vector.tensor_copy(outrep.rearrange("p c o -> p (c o)"), ps_rep)

        # iteration 0: uniform probs -> s = t0 / R
        nc.vector.tensor_copy(t_sb, ps_t0)
        nc.vector.memset(z_sb, float(R))
        squash(t_sb[:, :])

        for it in range(2):
            # delta pass: logits += sum_o priors * outrep
            for ch in range(NCH2):
                t0 = ch * TC2
                pr = ppool.tile([128, TC2, C, O], f16, tag="prod")
                nc.vector.tensor_mul(
                    pr,
                    priors[:, t0 : t0 + TC2, :, :],
                    outrep[:, None, :, :].broadcast_to([128, TC2, C, O]),
                )
                dl = dpool.tile([128, TC2, C], f32, tag="dl")
                nc.vector.tensor_reduce(dl, pr, axis=X, op=Alu.add)
                nc.vector.tensor_tensor(
                    out=logits[:, t0 : t0 + TC2, :],
                    in0=logits[:, t0 : t0 + TC2, :],
                    in1=dl,
                    op=Alu.add,
                )
            nc.scalar.activation(e_sb, logits, Act.Exp)
            # Z = sum_r e
            nc.vector.tensor_reduce(
                zpart, e_sb.transpose([0, 2, 1]), axis=X, op=Alu.add
            )
            ps_z = psum1.tile([8, C], f32, tag="z")
            nc.tensor.matmul(ps_z, sel8f, zpart)
            nc.vector.tensor_copy(z_sb, ps_z)
            # t = sum_r e * priors
            ps_t = psum1.tile([8, CO], f32, tag="t")
            for ch in range(NCH2):
                t0 = ch * TC2
                pr = ppool.tile([128, TC2, C, O], f16, tag="prod")
                nc.vector.tensor_mul(
                    pr,
                    priors[:, t0 : t0 + TC2, :, :],
                    e_sb[:, t0 : t0 + TC2, :, None].broadcast_to([128, TC2, C, O]),
                )
                for k in range(TC2):
                    t = t0 + k
                    nc.tensor.matmul(
                        ps_t,
                        sel8,
                        pr[:, k, :, :].rearrange("p c o -> p (c o)"),
                        start=(t == 0),
                        stop=(t == T - 1),
                        skip_group_check=True,
                    )
            nc.vector.tensor_copy(t_sb, ps_t)
            squash(t_sb[:, :])

        nc.sync.dma_start(
            out=out_d[:, :], in_=out_sb.rearrange("p c o -> p (c o)")
        )

    nc.compile()
    return nc


def _prep_inputs(x, route_weights):
    W = np.ascontiguousarray(route_weights)
    Wp = W.reshape(C, T, RP, I, O).transpose(1, 2, 3, 0, 4)
    Wp = np.ascontiguousarray(Wp, dtype=np.float16).reshape(T, 128, CO)
    xs = []
    for core in range(8):
        xl = x[core * NB : (core + 1) * NB]
        xp = xl.reshape(NB, T, RP, I).transpose(2, 3, 1, 0)
        xs.append(np.ascontiguousarray(xp, dtype=np.float16).reshape(128, T, NB))
    return Wp, xs


def kernel(x: np.ndarray, route_weights: np.ndarray) -> np.ndarray:
    from concourse.bass_utils import run_bass_kernel_spmd

    if "nc" not in _CACHE:
        _CACHE["nc"] = _build_program()
    nc = _CACHE["nc"]

    Wp, xs = _prep_inputs(
        np.asarray(x, np.float32), np.asarray(route_weights, np.float32)
    )
    in_maps = [{"w": Wp, "x": xs[core]} for core in range(8)]
    res = run_bass_kernel_spmd(
        nc,
        in_maps,
        core_ids=list(range(8)),
        trace=bool(int(os.environ.get("KTRACE", "0"))),
    )
    _CACHE["last_result"] = res
    outs = [res.results[core]["out"].reshape(NB, C, O) for core in range(8)]
    full = np.concatenate(outs, axis=0)
    out = full.transpose(1, 0, 2)[:, :, None, None, :]
    return np.ascontiguousarray(out.astype(np.float32))


if __name__ == "__main__":
    d = np.load("/root/problem/ref_data.npz")
    out = kernel(d["x"], d["W"])
    ref = d["ref"]
    err = float(np.abs(out - ref).max() / np.abs(ref).max())
    print("Relative error:", err)
